# revision 18
# baseline (speedup 1.0000x reference)
"""Multi-head attention (B=2, S=2048, D=1024, H=16, dh=64) on 8 TRN2 NeuronCores.

Sharding: tensor-parallel over heads - 2 heads per core. Each core computes
Q/K/V projections for its 2 heads, full attention over S=2048, and a partial
output projection (its 128 rows of Wo). Host sums the 8 partial outputs + bo.

v1 optimizations over the 424us baseline:
  - bf16 matmul operands everywhere (same 1 cyc/row PE rate, half the DMA)
  - x streamed as [128, 2048] bf16 tiles (4KB/partition DMA lines)
  - software-pipelined issue order: stage A for batch 1 and stage D/rb of the
    previous chunk are interleaved into the next chunk's B/exp/C stream so the
    PE never sits behind the softmax-normalize latency chain
  - normalize: DVE reciprocal_approx_fast on a [2,512] tile (one op per chunk)
    + PE ones-column matmul broadcast of 1/l (no DRAM round trip)
  - partial outputs written in bf16 (host accumulates in fp64)

Per-core dataflow:
  A) QKV:    psum[dh2=128, tok 512] = sum_k W_k[128,128].T @ x_k[128,512]
  T) V^T -> V via PE transpose (ctx matmul needs t on partitions)
  B) scoresT: psum[t=128, s 512] = K^T_h[64,128].T @ Q^T_h[64,512] (2 heads
     row-tiled, concurrent in the PE array)
  E) expT = exp(0.125 * scoresT) -> bf16 (ACT, scale folded; no max-subtract -
     scores are O(1) by construction)
  C) ctx aug: psum[65, 512] = sum_t [V_h|1][128,65].T @ expT[128,512]
     row 64 = softmax denominator l
  N) r = recip(l) [2,512]; rb psum[128,512] = ones[1,64].T @ r_h[1,512] (PE
     broadcast, h0 rows 0:64, h1 rows 64:128); ctxn = ctx * rb -> bf16
  D) out[s 128, d 512] = ctxn[:,s128][128,128].T @ Wo[128,512] -> bf16
"""

import numpy as np
import ml_dtypes

import concourse.bacc as bacc
import concourse.mybir as mybir
import concourse.tile as tile
from concourse.bass_utils import run_bass_kernel_spmd

F32 = mybir.dt.float32
F32R = mybir.dt.float32r
BF16 = mybir.dt.bfloat16

B, S, D, H, DH = 2, 2048, 1024, 16, 64
TOK = B * S          # 4096
DH2 = 2 * DH         # 128 (two heads per core)
NCORES = 8
SC = 512             # s-chunk
NSC = S // SC        # 4 s-chunks per batch
NT = S // 128        # 16 t-tiles per batch
NKT = D // 128       # 8 k-tiles of contraction
NCH = TOK // SC      # 8 token chunks for stage A


def build_bass():
    nc = bacc.Bacc(None, target_bir_lowering=False)

    # x pre-packed on host: piece (b, kt, hf) -> [128, 1024] contiguous
    xp = nc.dram_tensor("xp", [2 * NKT * 2, 128, 2 * SC], BF16,
                        kind="ExternalInput")
    wqkv = nc.dram_tensor("wqkv", [128, 3, NKT, DH2], BF16,
                          kind="ExternalInput")
    bqkv = nc.dram_tensor("bqkv", [128, 3], F32, kind="ExternalInput")
    wo = nc.dram_tensor("wo", [DH2, D], BF16, kind="ExternalInput")
    ones = nc.dram_tensor("ones", [128, 32], F32, kind="ExternalInput")
    onesf = nc.dram_tensor("onesf", [1, 64], BF16, kind="ExternalInput")
    iden = nc.dram_tensor("iden", [128, 128], F32, kind="ExternalInput")
    out = nc.dram_tensor("out", [TOK, D], BF16, kind="ExternalOutput")

    with tile.TileContext(nc) as tc:
        with (
            tc.tile_pool(name="persist", bufs=1) as persist,
            # one buf per x piece: a dma_start must NEVER wait on a pool
            # slot (the wait would block the queue that pumps software-DMA
            # descriptors and stall all in-flight transfers)
            tc.tile_pool(name="xin", bufs=32) as xin,
            tc.tile_pool(name="exps", bufs=8) as exps,
            tc.tile_pool(name="work", bufs=2) as work,
            tc.tile_pool(name="ctxs", bufs=2) as ctxs,
            tc.tile_pool(name="ost", bufs=3) as ost,
            tc.tile_pool(name="ps_big", bufs=2, space="PSUM") as ps_big,
            tc.tile_pool(name="ps_ctx", bufs=2, space="PSUM") as ps_ctx,
            tc.tile_pool(name="ps_u", bufs=2, space="PSUM") as ps_u,
        ):
            # ---- constants / persistent tiles ----
            w_sb = persist.tile([128, 3, NKT, DH2], BF16, tag="w")
            nc.scalar.dma_start(out=w_sb[:], in_=wqkv[:, :, :, :])
            b_sb = persist.tile([128, 3], F32, tag="b")
            nc.gpsimd.dma_start(out=b_sb[:], in_=bqkv[:, :])
            wo_sb = persist.tile([128, D], BF16, tag="wo")
            nc.sync.dma_start(out=wo_sb[:], in_=wo[:, :])
            ident = persist.tile([128, 128], F32R, tag="id")
            nc.sync.dma_start(out=ident[:], in_=iden[:, :].bitcast(F32R))
            ones1 = persist.tile([1, 64], BF16, tag="o1")
            nc.gpsimd.dma_start(out=ones1[:], in_=onesf[:, :])

            qT = persist.tile([128, TOK], BF16, tag="qT")
            kT = persist.tile([128, TOK], BF16, tag="kT")
            vT = persist.tile([128, TOK], F32R, tag="vT")
            # V in [t, e] layout, 130 = [V_h0(64) | 1 | V_h1(64) | 1]
            v_sb = persist.tile([128, TOK // 128, 130], F32R, tag="v")
            import concourse.bass as bass_mod
            o1 = ones[0:1, 0:TOK // 128]
            ones_b = bass_mod.AP(tensor=o1.tensor, offset=o1.offset,
                                 ap=[[0, 128], [1, TOK // 128]]).bitcast(F32R)
            nc.gpsimd.dma_start(out=v_sb[:, :, 64], in_=ones_b)
            nc.gpsimd.dma_start(out=v_sb[:, :, 129], in_=ones_b)


            # ---------------- stage helpers ----------------
            xtiles = {}  # batch -> list of 8 [128, 2048] tiles

            def issue_x_dmas(b, engs):
                tiles = [[None, None] for _ in range(NKT)]
                i = 0
                # hf-major: chunks 0/1 of the batch only need hf=0 pieces
                for hf in range(2):
                    for kt in range(NKT):
                        x_t = xin.tile([128, 2 * SC], BF16, tag="x",
                                       name=f"x{b}_{kt}_{hf}")
                        piece = (b * NKT + kt) * 2 + hf
                        engs[i % len(engs)].dma_start(
                            out=x_t[:], in_=xp[piece]
                        )
                        i += 1
                        tiles[kt][hf] = x_t
                xtiles[b] = tiles

            def stage_a_proj(ch, p):
                """One projection (0=q,1=k,2=v) for token chunk ch."""
                b, cc = divmod(ch, NSC)
                c0 = ch * SC
                tiles = xtiles[b]
                dests = (qT, kT, vT)
                hf, off = divmod(cc * SC, 2 * SC)
                ps_p = ps_u.tile([128, SC], F32, tag="u")
                for kt in range(NKT):
                    nc.tensor.matmul(
                        ps_p[:],
                        w_sb[:, p, kt, :],
                        tiles[kt][hf][:, off:off + SC],
                        start=(kt == 0), stop=(kt == NKT - 1),
                    )
                nc.vector.tensor_scalar_add(
                    dests[p][:, c0:c0 + SC], ps_p[:], b_sb[:, p:p + 1]
                )

            def stage_a_chunk(ch):
                for p in range(3):
                    stage_a_proj(ch, p)

            def stage_t(b):
                """Transpose V^T -> v_sb for batch b."""
                for blk in range(b * NT, (b + 1) * NT):
                    ps_t = ps_u.tile([128, 128], F32R, tag="u")
                    nc.tensor.transpose(
                        ps_t[:], vT[:, blk * 128:(blk + 1) * 128], ident[:]
                    )
                    nc.vector.tensor_copy(v_sb[:, blk, 0:64], ps_t[:, 0:64])
                    nc.vector.tensor_copy(
                        v_sb[:, blk, 65:129], ps_t[:, 64:128]
                    )

            # deferred tail state from the previous chunk
            pend = {}

            def issue_pend_rb(st):
                """PE broadcast of r rows into ps_rb, then ctxn muls (DVE)."""
                ps_rb = ps_u.tile([128, SC], F32, tag="u")
                for h in range(2):
                    nc.tensor.matmul(
                        ps_rb[h * 64:(h + 1) * 64, :],
                        ones1[:],
                        st["r2"][h][:],
                        start=True, stop=True,
                    )
                rb_sb = work.tile([128, SC], F32, tag="rb")
                nc.vector.tensor_copy(rb_sb[:], ps_rb[:])
                ctxn = ctxs.tile([128, SC], BF16, tag="ctxn")
                for h in range(2):
                    nc.vector.tensor_mul(
                        ctxn[h * 64:(h + 1) * 64, :],
                        st["ps_c"][h][0:64, :],
                        rb_sb[h * 64:(h + 1) * 64, :],
                    )
                st["ctxn"] = ctxn

            def issue_pend_d(st, ss):
                """One s-subtile of the output projection of a pending chunk."""
                q0 = st["q0"]
                ctxn = st["ctxn"]
                o_sb = ost.tile([128, 1024], BF16, tag="o")
                for dc in range(2):
                    ps_o = ps_u.tile([128, SC], F32, tag="u")
                    nc.tensor.matmul(
                        ps_o[:],
                        ctxn[:, ss * 128:(ss + 1) * 128],
                        wo_sb[:, dc * SC:(dc + 1) * SC],
                        start=True, stop=True,
                    )
                    nc.vector.tensor_copy(o_sb[:, dc * SC:(dc + 1) * SC], ps_o[:])
                nc.gpsimd.dma_start(
                    out=out[q0 + ss * 128:q0 + (ss + 1) * 128, :], in_=o_sb[:]
                )

            def chunk_body(b, sc, fillers):
                """B + exp + C for chunk (b, sc); `fillers` is a list of
                callables issued early to give the PE independent work while
                the previous chunk's normalize chain drains."""
                q0 = b * S + sc * SC
                etiles = []
                for tt in range(NT):
                    t0 = b * S + tt * 128
                    ps_s = ps_big.tile([128, 1024], F32, tag="big")
                    nc.tensor.matmul(
                        ps_s[:, 0:SC],
                        kT[0:64, t0:t0 + 128],
                        qT[0:64, q0:q0 + SC],
                        start=True, stop=True,
                    )
                    nc.tensor.matmul(
                        ps_s[:, SC:2 * SC],
                        kT[64:128, t0:t0 + 128],
                        qT[64:128, q0:q0 + SC],
                        start=True, stop=True,
                    )
                    e_t = exps.tile([128, 1024], F32R, tag="e")
                    nc.scalar.activation(
                        e_t[:], ps_s[:],
                        mybir.ActivationFunctionType.Exp, scale=0.125,
                    )
                    etiles.append(e_t)
                    if tt == 3:
                        # PE filler + deferred previous-chunk work while the
                        # first exps are in flight
                        for f in fillers:
                            f()
                        if pend:
                            issue_pend_rb(pend["st"])
                            for ss in range(4):
                                issue_pend_d(pend["st"], ss)
                            pend.clear()
                        ps_c = [
                            ps_ctx.tile([65, SC], F32, tag="ctx",
                                        name=f"psc{h}")
                            for h in range(2)
                        ]
                    if tt >= 3:
                        lag = tt - 3  # C trails B/exp by 4 t-tiles
                        for h in range(2):
                            nc.tensor.matmul(
                                ps_c[h][:],
                                v_sb[:, b * NT + lag, h * 65:h * 65 + 65],
                                etiles[lag][:, h * SC:(h + 1) * SC],
                                start=(lag == 0), stop=False,
                            )
                for lag in range(NT - 3, NT):
                    for h in range(2):
                        nc.tensor.matmul(
                            ps_c[h][:],
                            v_sb[:, b * NT + lag, h * 65:h * 65 + 65],
                            etiles[lag][:, h * SC:(h + 1) * SC],
                            start=False, stop=(lag == NT - 1),
                        )
                # denominators -> reciprocal (issued now; consumed next chunk)
                r2 = []
                for h in range(2):
                    l2 = work.tile([1, SC], F32, tag=f"l2{h}", name=f"l2{h}")
                    nc.vector.tensor_copy(l2[:], ps_c[h][64:65, :])
                    r2h = work.tile([1, SC], F32, tag=f"r2{h}", name=f"r2{h}")
                    nc.vector.reciprocal_approx_fast(r2h[:], l2[:])
                    r2r = work.tile([1, SC], BF16, tag=f"r2r{h}", name=f"r2r{h}")
                    nc.vector.tensor_copy(r2r[:], r2h[:])
                    r2.append(r2r)
                pend["st"] = {"q0": q0, "ps_c": ps_c, "r2": r2}

            # ---------------- issue order ----------------
            # batch-0 x on all four queues (everything else is idle early);
            # batch-1 x only on sync/gpsimd so in-DMA dispatch never blocks
            # the exp (scalar) or elementwise (vector) queues mid-loop
            issue_x_dmas(0, [nc.sync, nc.gpsimd, nc.scalar])
            for ch in range(4):
                stage_a_chunk(ch)
            stage_t(0)
            issue_x_dmas(1, [nc.sync, nc.gpsimd])

            # batch-1 QKV spread as PE filler: k+v during b0 chunks (T(1) and
            # chunk(1,0) need them), q(1,sc) rides one chunk ahead of its use
            chunk_body(0, 0, [lambda: stage_a_proj(4, 1),
                              lambda: stage_a_proj(4, 2)])
            chunk_body(0, 1, [lambda: stage_a_proj(5, 1),
                              lambda: stage_a_proj(5, 2)])
            chunk_body(0, 2, [lambda: stage_a_proj(6, 1),
                              lambda: stage_a_proj(6, 2)])
            chunk_body(0, 3, [lambda: stage_a_proj(7, 1),
                              lambda: stage_a_proj(7, 2),
                              lambda: stage_a_proj(4, 0)])
            chunk_body(1, 0, [lambda: stage_t(1),
                              lambda: stage_a_proj(5, 0)])
            chunk_body(1, 1, [lambda: stage_a_proj(6, 0)])
            chunk_body(1, 2, [lambda: stage_a_proj(7, 0)])
            chunk_body(1, 3, [])
            # drain the last chunk's tail
            issue_pend_rb(pend["st"])
            for ss in range(4):
                issue_pend_d(pend["st"], ss)
            pend.clear()
    nc.finalize()
    return nc


_NC_CACHE = None


def make_in_maps(x, Wq, Wk, Wv, bq, bk, bv, Wo, bo=None):
    bf = ml_dtypes.bfloat16
    xT = x.reshape(TOK, D).T.astype(bf)  # [D, TOK]
    # piece (b, kt, hf) -> xT[kt*128:(kt+1)*128, b*2048+hf*1024 :+1024]
    xp = np.ascontiguousarray(
        xT.reshape(NKT, 128, 2, 2, 2 * SC).transpose(2, 0, 3, 1, 4)
        .reshape(2 * NKT * 2, 128, 2 * SC))

    def pack_w(w2):  # [D, DH2] -> [128, NKT, DH2]
        return w2.reshape(NKT, 128, DH2).transpose(1, 0, 2)

    in_maps = []
    for c in range(NCORES):
        h0 = 2 * c
        wq2 = np.concatenate([Wq[h0], Wq[h0 + 1]], axis=1)
        wk2 = np.concatenate([Wk[h0], Wk[h0 + 1]], axis=1)
        wv2 = np.concatenate([Wv[h0], Wv[h0 + 1]], axis=1)
        wqkv = np.ascontiguousarray(
            np.stack([pack_w(wq2), pack_w(wk2), pack_w(wv2)], axis=1)
        ).astype(bf)
        bqkv = np.ascontiguousarray(np.stack([
            bq[h0:h0 + 2].reshape(DH2),
            bk[h0:h0 + 2].reshape(DH2),
            bv[h0:h0 + 2].reshape(DH2),
        ], axis=1)).astype(np.float32)
        in_maps.append({
            "xp": xp,
            "wqkv": wqkv,
            "bqkv": bqkv,
            "wo": np.ascontiguousarray(Wo[c * DH2:(c + 1) * DH2]).astype(bf),
            "ones": np.ones((128, 32), dtype=np.float32),
            "onesf": np.ones((1, 64), dtype=bf),
            "iden": np.eye(128, dtype=np.float32),
        })
    return in_maps


def kernel(x, Wq, Wk, Wv, bq, bk, bv, Wo, bo):
    global _NC_CACHE
    if _NC_CACHE is None:
        _NC_CACHE = build_bass()
    nc = _NC_CACHE

    in_maps = make_in_maps(x, Wq, Wk, Wv, bq, bk, bv, Wo)
    res = run_bass_kernel_spmd(nc, in_maps, list(range(NCORES)))
    acc = np.zeros((TOK, D), dtype=np.float64)
    for c in range(NCORES):
        acc += res.results[c]["out"].astype(np.float64)
    acc += bo
    return acc.astype(np.float32).reshape(B, S, D)


# revision 19
# speedup vs baseline: 1.0628x; 1.0628x over previous
"""Multi-head attention (B=2, S=2048, D=1024, H=16, dh=64) on 8 TRN2 NeuronCores.

Sharding: tensor-parallel over heads - 2 heads per core. Each core computes
Q/K/V projections for its 2 heads, full attention over S=2048, and a partial
output projection (its 128 rows of Wo). Host sums the 8 partial outputs + bo.

v1 optimizations over the 424us baseline:
  - bf16 matmul operands everywhere (same 1 cyc/row PE rate, half the DMA)
  - x streamed as [128, 2048] bf16 tiles (4KB/partition DMA lines)
  - software-pipelined issue order: stage A for batch 1 and stage D/rb of the
    previous chunk are interleaved into the next chunk's B/exp/C stream so the
    PE never sits behind the softmax-normalize latency chain
  - normalize: DVE reciprocal_approx_fast on a [2,512] tile (one op per chunk)
    + PE ones-column matmul broadcast of 1/l (no DRAM round trip)
  - partial outputs written in bf16 (host accumulates in fp64)

Per-core dataflow:
  A) QKV:    psum[dh2=128, tok 512] = sum_k W_k[128,128].T @ x_k[128,512]
  T) V^T -> V via PE transpose (ctx matmul needs t on partitions)
  B) scoresT: psum[t=128, s 512] = K^T_h[64,128].T @ Q^T_h[64,512] (2 heads
     row-tiled, concurrent in the PE array)
  E) expT = exp(0.125 * scoresT) -> bf16 (ACT, scale folded; no max-subtract -
     scores are O(1) by construction)
  C) ctx aug: psum[65, 512] = sum_t [V_h|1][128,65].T @ expT[128,512]
     row 64 = softmax denominator l
  N) r = recip(l) [2,512]; rb psum[128,512] = ones[1,64].T @ r_h[1,512] (PE
     broadcast, h0 rows 0:64, h1 rows 64:128); ctxn = ctx * rb -> bf16
  D) out[s 128, d 512] = ctxn[:,s128][128,128].T @ Wo[128,512] -> bf16
"""

import numpy as np
import ml_dtypes

import concourse.bacc as bacc
import concourse.mybir as mybir
import concourse.tile as tile
from concourse.bass_utils import run_bass_kernel_spmd

F32 = mybir.dt.float32
F32R = mybir.dt.float32r
BF16 = mybir.dt.bfloat16

B, S, D, H, DH = 2, 2048, 1024, 16, 64
TOK = B * S          # 4096
DH2 = 2 * DH         # 128 (two heads per core)
NCORES = 8
SC = 512             # s-chunk
NSC = S // SC        # 4 s-chunks per batch
NT = S // 128        # 16 t-tiles per batch
NKT = D // 128       # 8 k-tiles of contraction
NCH = TOK // SC      # 8 token chunks for stage A


def build_bass():
    nc = bacc.Bacc(None, target_bir_lowering=False)

    # x pre-packed on host: piece (b, kt, hf) -> [128, 1024] contiguous
    xp = nc.dram_tensor("xp", [2 * NKT * 2, 128, 2 * SC], BF16,
                        kind="ExternalInput")
    wqkv = nc.dram_tensor("wqkv", [128, 3, NKT, DH2], BF16,
                          kind="ExternalInput")
    bqkv = nc.dram_tensor("bqkv", [128, 3], F32, kind="ExternalInput")
    wo = nc.dram_tensor("wo", [DH2, D], BF16, kind="ExternalInput")
    ones = nc.dram_tensor("ones", [128, 32], F32, kind="ExternalInput")
    onesf = nc.dram_tensor("onesf", [1, 64], BF16, kind="ExternalInput")
    iden = nc.dram_tensor("iden", [128, 128], F32, kind="ExternalInput")
    out = nc.dram_tensor("out", [TOK, D], BF16, kind="ExternalOutput")

    with tile.TileContext(nc) as tc:
        with (
            tc.tile_pool(name="persist", bufs=1) as persist,
            # one buf per x piece: a dma_start must NEVER wait on a pool
            # slot (the wait would block the queue that pumps software-DMA
            # descriptors and stall all in-flight transfers)
            tc.tile_pool(name="xin", bufs=32) as xin,
            tc.tile_pool(name="exps", bufs=8) as exps,
            tc.tile_pool(name="work", bufs=2) as work,
            tc.tile_pool(name="ctxs", bufs=2) as ctxs,
            tc.tile_pool(name="ost", bufs=3) as ost,
            tc.tile_pool(name="ps_big", bufs=2, space="PSUM") as ps_big,
            tc.tile_pool(name="ps_ctx", bufs=2, space="PSUM") as ps_ctx,
            tc.tile_pool(name="ps_u", bufs=2, space="PSUM") as ps_u,
        ):
            # ---- constants / persistent tiles ----
            w_sb = persist.tile([128, 3, NKT, DH2], BF16, tag="w")
            nc.scalar.dma_start(out=w_sb[:], in_=wqkv[:, :, :, :])
            b_sb = persist.tile([128, 3], F32, tag="b")
            nc.gpsimd.dma_start(out=b_sb[:], in_=bqkv[:, :])
            wo_sb = persist.tile([128, D], BF16, tag="wo")
            nc.sync.dma_start(out=wo_sb[:], in_=wo[:, :])
            ident = persist.tile([128, 128], F32R, tag="id")
            nc.sync.dma_start(out=ident[:], in_=iden[:, :].bitcast(F32R))
            ones1 = persist.tile([1, 64], BF16, tag="o1")
            nc.gpsimd.dma_start(out=ones1[:], in_=onesf[:, :])

            qT = persist.tile([128, TOK], BF16, tag="qT")
            kT = persist.tile([128, TOK], BF16, tag="kT")
            vT = persist.tile([128, TOK], F32R, tag="vT")
            # V in [t, e] layout, 130 = [V_h0(64) | 1 | V_h1(64) | 1]
            v_sb = persist.tile([128, TOK // 128, 130], F32R, tag="v")
            import concourse.bass as bass_mod
            o1 = ones[0:1, 0:TOK // 128]
            ones_b = bass_mod.AP(tensor=o1.tensor, offset=o1.offset,
                                 ap=[[0, 128], [1, TOK // 128]]).bitcast(F32R)
            nc.gpsimd.dma_start(out=v_sb[:, :, 64], in_=ones_b)
            nc.gpsimd.dma_start(out=v_sb[:, :, 129], in_=ones_b)


            # ---------------- stage helpers ----------------
            xtiles = {}  # batch -> list of 8 [128, 2048] tiles

            def issue_x_dmas(b, engs):
                tiles = [[None, None] for _ in range(NKT)]
                i = 0
                # hf-major: chunks 0/1 of the batch only need hf=0 pieces
                for hf in range(2):
                    for kt in range(NKT):
                        x_t = xin.tile([128, 2 * SC], BF16, tag="x",
                                       name=f"x{b}_{kt}_{hf}")
                        piece = (b * NKT + kt) * 2 + hf
                        engs[i % len(engs)].dma_start(
                            out=x_t[:], in_=xp[piece]
                        )
                        i += 1
                        tiles[kt][hf] = x_t
                xtiles[b] = tiles

            def stage_a_proj(ch, p):
                """One projection (0=q,1=k,2=v) for token chunk ch."""
                b, cc = divmod(ch, NSC)
                c0 = ch * SC
                tiles = xtiles[b]
                dests = (qT, kT, vT)
                hf, off = divmod(cc * SC, 2 * SC)
                ps_p = ps_u.tile([128, SC], F32, tag="u")
                for kt in range(NKT):
                    nc.tensor.matmul(
                        ps_p[:],
                        w_sb[:, p, kt, :],
                        tiles[kt][hf][:, off:off + SC],
                        start=(kt == 0), stop=(kt == NKT - 1),
                    )
                nc.vector.tensor_scalar_add(
                    dests[p][:, c0:c0 + SC], ps_p[:], b_sb[:, p:p + 1]
                )

            def stage_a_chunk(ch):
                for p in range(3):
                    stage_a_proj(ch, p)

            def stage_t(b):
                """Transpose V^T -> v_sb for batch b."""
                for blk in range(b * NT, (b + 1) * NT):
                    ps_t = ps_u.tile([128, 128], F32R, tag="u")
                    nc.tensor.transpose(
                        ps_t[:], vT[:, blk * 128:(blk + 1) * 128], ident[:]
                    )
                    nc.vector.tensor_copy(v_sb[:, blk, 0:64], ps_t[:, 0:64])
                    nc.vector.tensor_copy(
                        v_sb[:, blk, 65:129], ps_t[:, 64:128]
                    )

            # deferred tail state from the previous chunk
            pend = {}

            def issue_pend_rb(st):
                """PE broadcast of r rows into ps_rb, then ctxn muls (DVE)."""
                ps_rb = ps_u.tile([128, SC], F32, tag="u")
                for h in range(2):
                    nc.tensor.matmul(
                        ps_rb[h * 64:(h + 1) * 64, :],
                        ones1[:],
                        st["r2"][h][:],
                        start=True, stop=True,
                    )
                rb_sb = work.tile([128, SC], F32, tag="rb")
                nc.vector.tensor_copy(rb_sb[:], ps_rb[:])
                ctxn = ctxs.tile([128, SC], BF16, tag="ctxn")
                for h in range(2):
                    nc.vector.tensor_mul(
                        ctxn[h * 64:(h + 1) * 64, :],
                        st["ps_c"][h][0:64, :],
                        rb_sb[h * 64:(h + 1) * 64, :],
                    )
                st["ctxn"] = ctxn

            def issue_pend_d(st, ss):
                """One s-subtile of the output projection of a pending chunk."""
                q0 = st["q0"]
                ctxn = st["ctxn"]
                o_sb = ost.tile([128, 1024], BF16, tag="o")
                for dc in range(2):
                    ps_o = ps_u.tile([128, SC], F32, tag="u")
                    nc.tensor.matmul(
                        ps_o[:],
                        ctxn[:, ss * 128:(ss + 1) * 128],
                        wo_sb[:, dc * SC:(dc + 1) * SC],
                        start=True, stop=True,
                    )
                    nc.vector.tensor_copy(o_sb[:, dc * SC:(dc + 1) * SC], ps_o[:])
                nc.sync.dma_start(
                    out=out[q0 + ss * 128:q0 + (ss + 1) * 128, :], in_=o_sb[:]
                )

            def chunk_body(b, sc, fillers):
                """B + exp + C for chunk (b, sc); `fillers` is a list of
                callables issued early to give the PE independent work while
                the previous chunk's normalize chain drains."""
                q0 = b * S + sc * SC
                etiles = []
                for tt in range(NT):
                    t0 = b * S + tt * 128
                    ps_s = ps_big.tile([128, 1024], F32, tag="big")
                    nc.tensor.matmul(
                        ps_s[:, 0:SC],
                        kT[0:64, t0:t0 + 128],
                        qT[0:64, q0:q0 + SC],
                        start=True, stop=True,
                    )
                    nc.tensor.matmul(
                        ps_s[:, SC:2 * SC],
                        kT[64:128, t0:t0 + 128],
                        qT[64:128, q0:q0 + SC],
                        start=True, stop=True,
                    )
                    e_t = exps.tile([128, 1024], F32R, tag="e")
                    nc.scalar.activation(
                        e_t[:], ps_s[:],
                        mybir.ActivationFunctionType.Exp, scale=0.125,
                    )
                    etiles.append(e_t)
                    if tt == 3:
                        # PE filler + deferred previous-chunk work while the
                        # first exps are in flight
                        for f in fillers:
                            f()
                        if pend:
                            issue_pend_rb(pend["st"])
                            for ss in range(4):
                                issue_pend_d(pend["st"], ss)
                            pend.clear()
                        ps_c = [
                            ps_ctx.tile([65, SC], F32, tag="ctx",
                                        name=f"psc{h}")
                            for h in range(2)
                        ]
                    if tt >= 3:
                        lag = tt - 3  # C trails B/exp by 4 t-tiles
                        for h in range(2):
                            nc.tensor.matmul(
                                ps_c[h][:],
                                v_sb[:, b * NT + lag, h * 65:h * 65 + 65],
                                etiles[lag][:, h * SC:(h + 1) * SC],
                                start=(lag == 0), stop=False,
                            )
                for lag in range(NT - 3, NT):
                    for h in range(2):
                        nc.tensor.matmul(
                            ps_c[h][:],
                            v_sb[:, b * NT + lag, h * 65:h * 65 + 65],
                            etiles[lag][:, h * SC:(h + 1) * SC],
                            start=False, stop=(lag == NT - 1),
                        )
                # denominators -> reciprocal (issued now; consumed next chunk)
                r2 = []
                for h in range(2):
                    l2 = work.tile([1, SC], F32, tag=f"l2{h}", name=f"l2{h}")
                    nc.vector.tensor_copy(l2[:], ps_c[h][64:65, :])
                    r2h = work.tile([1, SC], F32, tag=f"r2{h}", name=f"r2{h}")
                    nc.vector.reciprocal_approx_fast(r2h[:], l2[:])
                    r2r = work.tile([1, SC], BF16, tag=f"r2r{h}", name=f"r2r{h}")
                    nc.vector.tensor_copy(r2r[:], r2h[:])
                    r2.append(r2r)
                pend["st"] = {"q0": q0, "ps_c": ps_c, "r2": r2}

            # ---------------- issue order ----------------
            # batch-0 x on all four queues (everything else is idle early);
            # batch-1 x only on sync/gpsimd so in-DMA dispatch never blocks
            # the exp (scalar) or elementwise (vector) queues mid-loop
            issue_x_dmas(0, [nc.sync, nc.scalar])
            for ch in range(4):
                stage_a_chunk(ch)
            stage_t(0)
            issue_x_dmas(1, [nc.sync, nc.scalar])

            # batch-1 QKV spread as PE filler: k+v during b0 chunks (T(1) and
            # chunk(1,0) need them), q(1,sc) rides one chunk ahead of its use
            chunk_body(0, 0, [lambda: stage_a_proj(4, 1),
                              lambda: stage_a_proj(4, 2)])
            chunk_body(0, 1, [lambda: stage_a_proj(5, 1),
                              lambda: stage_a_proj(5, 2)])
            chunk_body(0, 2, [lambda: stage_a_proj(6, 1),
                              lambda: stage_a_proj(6, 2)])
            chunk_body(0, 3, [lambda: stage_a_proj(7, 1),
                              lambda: stage_a_proj(7, 2),
                              lambda: stage_a_proj(4, 0)])
            chunk_body(1, 0, [lambda: stage_t(1),
                              lambda: stage_a_proj(5, 0)])
            chunk_body(1, 1, [lambda: stage_a_proj(6, 0)])
            chunk_body(1, 2, [lambda: stage_a_proj(7, 0)])
            chunk_body(1, 3, [])
            # drain the last chunk's tail
            issue_pend_rb(pend["st"])
            for ss in range(4):
                issue_pend_d(pend["st"], ss)
            pend.clear()
    nc.finalize()
    return nc


_NC_CACHE = None


def make_in_maps(x, Wq, Wk, Wv, bq, bk, bv, Wo, bo=None):
    bf = ml_dtypes.bfloat16
    xT = x.reshape(TOK, D).T.astype(bf)  # [D, TOK]
    # piece (b, kt, hf) -> xT[kt*128:(kt+1)*128, b*2048+hf*1024 :+1024]
    xp = np.ascontiguousarray(
        xT.reshape(NKT, 128, 2, 2, 2 * SC).transpose(2, 0, 3, 1, 4)
        .reshape(2 * NKT * 2, 128, 2 * SC))

    def pack_w(w2):  # [D, DH2] -> [128, NKT, DH2]
        return w2.reshape(NKT, 128, DH2).transpose(1, 0, 2)

    in_maps = []
    for c in range(NCORES):
        h0 = 2 * c
        wq2 = np.concatenate([Wq[h0], Wq[h0 + 1]], axis=1)
        wk2 = np.concatenate([Wk[h0], Wk[h0 + 1]], axis=1)
        wv2 = np.concatenate([Wv[h0], Wv[h0 + 1]], axis=1)
        wqkv = np.ascontiguousarray(
            np.stack([pack_w(wq2), pack_w(wk2), pack_w(wv2)], axis=1)
        ).astype(bf)
        bqkv = np.ascontiguousarray(np.stack([
            bq[h0:h0 + 2].reshape(DH2),
            bk[h0:h0 + 2].reshape(DH2),
            bv[h0:h0 + 2].reshape(DH2),
        ], axis=1)).astype(np.float32)
        in_maps.append({
            "xp": xp,
            "wqkv": wqkv,
            "bqkv": bqkv,
            "wo": np.ascontiguousarray(Wo[c * DH2:(c + 1) * DH2]).astype(bf),
            "ones": np.ones((128, 32), dtype=np.float32),
            "onesf": np.ones((1, 64), dtype=bf),
            "iden": np.eye(128, dtype=np.float32),
        })
    return in_maps


def kernel(x, Wq, Wk, Wv, bq, bk, bv, Wo, bo):
    global _NC_CACHE
    if _NC_CACHE is None:
        _NC_CACHE = build_bass()
    nc = _NC_CACHE

    in_maps = make_in_maps(x, Wq, Wk, Wv, bq, bk, bv, Wo)
    res = run_bass_kernel_spmd(nc, in_maps, list(range(NCORES)))
    acc = np.zeros((TOK, D), dtype=np.float64)
    for c in range(NCORES):
        acc += res.results[c]["out"].astype(np.float64)
    acc += bo
    return acc.astype(np.float32).reshape(B, S, D)
